# revision 30
# baseline (speedup 1.0000x reference)
"""Trainium2 Bass kernel for nn_ErrorCorrectionModule (vq_codebook).

v3: fp8e4 + DoubleRow matmuls everywhere except the error gate.

Sharding (unchanged from v2): core c owns global heads {2c, 2c+1} for BOTH
batches at full sequence length, so Q/K/V projections need no collective
(host-sliced weight columns).  After attention the per-head contexts are
exchanged with two 8-core AllToAlls whose shard j carries tokens of core j
(b=j//4, blk=j%4).  The memory softmax, error gate and the fused (Wo@W1)
output matmul + LN run token-parallel on each core's own 512 tokens.

v3 changes:
 - All big matmuls are fp8e4 with MatmulPerfMode.DoubleRow, pairing two
   128-deep contraction chunks per pass (projections, ctx, memory, W1).
   Scores (64-deep contraction) use a stride-0 pair dim on both operands,
   which doubles the result (folded into the exp scale).
 - Weights are pre-scaled by powers of 2 on the host so fp8 operands sit in
   a healthy range: wq/wk/wv/em/emT/w1m x32, wf x64.  All PSUM accumulation
   groups in the W1 stage share product scale 1024, divided out during PSUM
   evacuation.
 - Softmax is normalized on the SENDER (ones-column of V carries 2*sum(p);
   a ones-matmul broadcasts 1/Z across partitions), so the AllToAll payload
   is normalized fp8 ctx (x16), 64 rows per head, no Z row, and the receiver
   does no PE work.  Scale order s2, s4 (ship AllToAll A), then s1 (ship B)
   so the big scale-1 exchange departs as early as possible.
 - Host inputs are pre-chunked [128, 8, N] so every DMA reads >=1KB
   contiguous runs; all input loads ride the sync queue in priority order.
 - The gate path stays bf16 (it is a graded output).

This walrus build accepts at most one sync wait per instruction, so a
post-pass (_split_waits) parks Tile's extra waits on standalone
EventSemaphore carriers; the Tile kernel-tail drain gets the same treatment
(_SplitDrainTileContext).
"""
import numpy as np
import ml_dtypes

import concourse.bass as bass
import concourse.tile as tile
import concourse.mybir as mybir
from concourse.bass import ds
from concourse.bass_utils import run_bass_kernel_spmd

B, L, H = 2, 2048, 1024
NH, HD = 16, 64
M = 1024
SCALES = (1, 2, 4)
EPS = 1e-6
NCORES = 8
GROUP = 4
TPC = L // GROUP   # 512 own tokens (outputs / memory / gate)
HPC = 2            # head-pairs per core: heads {2c, 2c+1}
FPC = HPC * HD     # 128 features per core

F32 = mybir.dt.float32
BF16 = mybir.dt.bfloat16
FP8 = mybir.dt.float8e4
F32R = mybir.dt.float32r
AF = mybir.ActivationFunctionType
ALU = mybir.AluOpType
DR = mybir.MatmulPerfMode.DoubleRow
BF = ml_dtypes.bfloat16
E4 = ml_dtypes.float8_e4m3

SW = 32.0            # fp8 weight upscale (wq/wk/wv/em/emT/w1m)
SHIP = 16.0          # shipped ctx scale (V ones col = 32/16 = 2)
PS_DESCALE = 1.0 / 1024.0   # W1-stage PSUM product scale (16*64 = 32*32)
EXP_SCALE_ATT = 0.125 / 2048.0   # 1/8 true scale; PSUM = 2*32*32*(q.k)
EXP_SCALE_MEM = 1.0 / 1024.0     # 1/32 true scale; PSUM = 32*(x.em)

# AllToAll A packs scales 2 and 4: per-shard element offsets (fp8 elems)
_SHA_S2 = HPC * 64 * (TPC // 2)          # scale-2 block: 2*64*256
_SHA = _SHA_S2 + HPC * 64 * (TPC // 4)   # + scale-4 block: 2*64*128


class _SplitDrainTileContext(tile.TileContext):
    """Kernel-tail drain must carry <=2 sync waits on this walrus build; park
    the waits on standalone single-wait EventSemaphore instructions."""

    def _drain_and_barrier(self, tick_clock, wait_clock):
        from concourse.vector_clock import ScopedClock
        nc = self.nc
        probe = nc.sync.drain()
        wait_clock.add_sem_waits(
            probe.ins, ScopedClock({None: tick_clock.global_clock}))
        si = probe.ins.sync_info
        waits = list(si.on_wait) if si is not None and si.on_wait else []
        if si is not None and waits:
            si.on_wait = []
            probe.ins.sync_info = si
        sem_by_num = {h.num: h for h in self.sems.allocated().values()}
        for w in waits:
            nc.sync.wait_ge(sem_by_num[w.id], w.wait_value)
        nc.sync.drain()
        nc.all_engine_barrier()
        assert self.sems is not None
        popped = nc._tile_sem_poison_stack.pop()
        assert popped is self._sem_poison
        nc.clear_and_free_semaphores(list(self.sems.allocated().values()))
        nc.all_engine_barrier()


def _split_waits(nc, dma_limit=1, other_limit=1):
    """Walrus on this stack accepts at most 1 sync wait per instruction
    (2 on EventSemaphore); Tile attaches as many as deps require. Move the
    excess onto standalone EventSemaphore carriers inserted just before."""
    ctr = 0
    for f in nc.m.functions:
        for bb in f.blocks:
            out = []
            changed = False
            for inst in bb.instructions:
                si = inst.sync_info
                waits = list(si.on_wait) if si is not None and si.on_wait else []
                tn = type(inst).__name__
                if "EventSemaphore" in tn:
                    limit = 2
                elif "DMA" in tn:
                    limit = dma_limit
                else:
                    limit = other_limit
                if len(waits) > limit:
                    excess = waits[:len(waits) - limit]
                    keep = waits[len(waits) - limit:]
                    for i in range(0, len(excess), 2):
                        ev = mybir.InstEventSemaphore(
                            name=f"WS-{ctr}", ins=[], outs=[])
                        ctr += 1
                        ev.engine = inst.engine
                        ev.sync_info = mybir.SyncInfo(
                            on_wait=excess[i:i + 2], on_update=[])
                        nc.register_instruction(ev, overwrite=True)
                        out.append(ev)
                    si.on_wait = keep
                    inst.sync_info = si
                    changed = True
                out.append(inst)
            if changed:
                bb.instructions = out
    return nc


def _pair0(ap):
    """Insert a stride-0 [0, 2] pair dim right after the partition dim."""
    return bass.AP(tensor=ap.tensor, offset=ap.offset,
                   ap=[list(ap.ap[0])] + [[0, 2]] + [list(p) for p in ap.ap[1:]])


def _build_program():
    nc = bass.Bass()

    xt = nc.declare_dram_parameter("xt", [TPC, H], F32, isOutput=False)
    xToT = nc.declare_dram_parameter("xToT", [128, 8, TPC], BF16, isOutput=False)
    xToT8 = nc.declare_dram_parameter("xToT8", [128, 8, TPC], FP8, isOutput=False)
    x1T = nc.declare_dram_parameter("x1T", [128, 8, B, L], FP8, isOutput=False)
    x2T = nc.declare_dram_parameter("x2T", [128, 8, B, L // 2], FP8,
                                    isOutput=False)
    x4T = nc.declare_dram_parameter("x4T", [128, 8, B, L // 4], FP8,
                                    isOutput=False)
    wq = nc.declare_dram_parameter("wq", [3, 128, 8, FPC], FP8, isOutput=False)
    wk = nc.declare_dram_parameter("wk", [3, 128, 8, FPC], FP8, isOutput=False)
    wv = nc.declare_dram_parameter("wv", [3, 128, 8, FPC], FP8, isOutput=False)
    wf = nc.declare_dram_parameter("wf", [3, 128, 8, H], FP8, isOutput=False)
    w1m = nc.declare_dram_parameter("w1m", [128, 8, H], FP8, isOutput=False)
    w2 = nc.declare_dram_parameter("w2", [128, 8, H], BF16, isOutput=False)
    emT = nc.declare_dram_parameter("emT", [128, 8, 8, 128], FP8, isOutput=False)
    em = nc.declare_dram_parameter("em", [128, 8, 8, 128], FP8, isOutput=False)
    zsel = nc.declare_dram_parameter("zsel", [128, 64], FP8, isOutput=False)
    bqs = nc.declare_dram_parameter("bqs", [128, 3], F32, isOutput=False)
    bvf = nc.declare_dram_parameter("bvf", [3, FPC], F32, isOutput=False)
    vecs = nc.declare_dram_parameter("vecs", [4, H], F32, isOutput=False)
    vecsb = nc.declare_dram_parameter("vecsb", [3, H], BF16, isOutput=False)
    onesr = nc.declare_dram_parameter("onesr", [1, 128], F32R, isOutput=False)
    c8 = nc.declare_dram_parameter("c8", [128, 3], FP8, isOutput=False)

    out_y = nc.declare_dram_parameter("out_y", [TPC, H], BF16, isOutput=True)
    out_g = nc.declare_dram_parameter("out_g", [TPC, H], F32, isOutput=True)

    def rep_vec(row):
        v = vecs[row, :]
        return bass.AP(tensor=v.tensor, offset=v.offset,
                       ap=[[0, 128]] + [list(p) for p in v.ap])

    with _SplitDrainTileContext(nc) as tc:
        with (
            tc.tile_pool(name="const", bufs=1) as const,
            tc.tile_pool(name="dram", bufs=1, space="DRAM") as dram,
            tc.tile_pool(name="ppmm", bufs=2, space="PSUM") as ppmm,
        ):
            # ---- tiny consts (Activation queue: keep the sync queue free
            # for the attention-critical loads; Act is idle at startup) ----
            onesr_sb = const.tile([1, 128], F32R)
            nc.scalar.dma_start(out=onesr_sb, in_=onesr[:, :])
            epst = const.tile([128, 1], F32)
            nc.vector.memset(epst, EPS)
            bq_sb = const.tile([128, 3], F32)
            nc.scalar.dma_start(out=bq_sb, in_=bqs[:, :])
            bvf_sb = const.tile([128, 3, FPC], F32)
            _bv = bvf[:, :]
            nc.scalar.dma_start(
                out=bvf_sb,
                in_=bass.AP(tensor=_bv.tensor, offset=_bv.offset,
                            ap=[[0, 128]] + [list(p) for p in _bv.ap]))

            a2aA_in = dram.tile([NCORES, _SHA], FP8, name="a2aAi", tag="a2aAi")
            a2aA_out = dram.tile([NCORES, _SHA], FP8, name="a2aAo", tag="a2aAo")
            a2aB_in = dram.tile([NCORES, HPC, 64, TPC], FP8,
                                name="a2aBi", tag="a2aBi")
            a2aB_out = dram.tile([NCORES, HPC, 64, TPC], FP8,
                                 name="a2aBo", tag="a2aBo")

            # ---- late-phase persistent tiles (pool stays open past the
            # attention scopes; closed explicitly before context exit) ----
            late_cm = tc.tile_pool(name="late", bufs=1)
            late = late_cm.__enter__()
            gate_bf = late.tile([128, 4, H], BF16)
            sumg = late.tile([128, 4], F32)
            mcT8 = late.tile([128, 4, 8, 128], FP8)
            ctxU = {
                1: late.tile([128, 4, 8, 128], FP8, name="ctxU1", tag="ctxU1"),
                2: late.tile([128, 4, 8, 128], FP8, name="ctxU2", tag="ctxU2"),
                4: late.tile([128, 4, 8, 128], FP8, name="ctxU4", tag="ctxU4"),
            }
            wf_sb = [late.tile([128, 8, H], FP8, name=f"wf{i}", tag=f"wf{i}")
                     for i in range(3)]
            w1m_sb = late.tile([128, 8, H], FP8, name="w1ms", tag="w1ms")
            w2_sb = late.tile([128, 8, H], BF16, name="w2s", tag="w2s")
            em_sb = late.tile([128, 8, 8, 128], FP8, name="ems", tag="ems")
            emT_sb = late.tile([128, 8, 8, 128], FP8, name="emTs", tag="emTs")
            zsel_sb = late.tile([128, 2, 32], FP8, name="zsel", tag="zsel")
            xTo_sb = late.tile([128, 8, TPC], BF16, name="xTo", tag="xTo")
            xTo8_sb = late.tile([128, 8, TPC], FP8, name="xTo8", tag="xTo8")

            # ---------- attention ----------
            with tc.tile_pool(name="wpsc", bufs=1) as wpsc, \
                 tc.tile_pool(name="xp1", bufs=1) as xp1, \
                 tc.tile_pool(name="ppsp", bufs=2, space="PSUM") as ppsp, \
                 tc.tile_pool(name="ppctx", bufs=2, space="PSUM") as ppctx:
                qkv_w = {}
                for s in SCALES:
                    ws = {}
                    for nm, wt in (("wk", wk), ("wq", wq), ("wv", wv)):
                        ws[nm] = wpsc.tile([128, 8, FPC], FP8,
                                           name=f"{nm}{s}", tag=f"{nm}{s}")
                    qkv_w[s] = ws
                # priority DMA order on the sync queue
                for nm, wt in (("wk", wk), ("wq", wq), ("wv", wv)):
                    nc.sync.dma_start(out=qkv_w[2][nm], in_=wt[1, :, :, :])
                xs_sb = {}
                xs_sb[2] = xp1.tile([128, 8, B, L // 2], FP8,
                                    name="xs2", tag="xs2")
                for half in range(2):
                    nc.sync.dma_start(
                        out=xs_sb[2][:, :, 0, ds(half * 512, 512)],
                        in_=x2T[:, :, 0, ds(half * 512, 512)])
                nc.sync.dma_start(out=xs_sb[2][:, :, 1, :],
                                  in_=x2T[:, :, 1, :])
                for nm, wt in (("wk", wk), ("wq", wq), ("wv", wv)):
                    nc.sync.dma_start(out=qkv_w[4][nm], in_=wt[2, :, :, :])
                    nc.sync.dma_start(out=qkv_w[1][nm], in_=wt[0, :, :, :])
                xs_sb[4] = xp1.tile([128, 8, B, L // 4], FP8,
                                    name="xs4", tag="xs4")
                nc.sync.dma_start(out=xs_sb[4], in_=x4T[:, :, :, :])
                xs_sb[1] = xp1.tile([128, 8, B, L], FP8, name="xs1", tag="xs1")
                for bb in range(B):
                    for half in range(2):
                        nc.sync.dma_start(
                            out=xs_sb[1][:, :, bb, ds(half * (L // 2), L // 2)],
                            in_=x1T[:, :, bb, ds(half * (L // 2), L // 2)])
                # late weights, chunked so urgent DMAs never wait long
                for i in range(3):
                    for hc in range(0, 8, 4):
                        nc.sync.dma_start(out=wf_sb[i][:, ds(hc, 4), :],
                                          in_=wf[i, :, ds(hc, 4), :])
                nc.sync.dma_start(out=w1m_sb, in_=w1m[:, :, :])
                nc.sync.dma_start(out=xTo_sb, in_=xToT[:, :, :])
                nc.sync.dma_start(out=xTo8_sb, in_=xToT8[:, :, :])
                for hc in range(0, 8, 4):
                    nc.sync.dma_start(out=w2_sb[:, ds(hc, 4), :],
                                      in_=w2[:, ds(hc, 4), :])
                nc.sync.dma_start(out=emT_sb, in_=emT[:, :, :, :])
                nc.sync.dma_start(out=em_sb, in_=em[:, :, :, :])
                nc.sync.dma_start(
                    out=zsel_sb,
                    in_=zsel.rearrange("p (two c) -> p two c", two=2))

                st = {}
                awork_cm = tc.tile_pool(name="awork", bufs=2)
                awork = awork_cm.__enter__()
                nwork_cm = tc.tile_pool(name="nwork", bufs=2)
                nwork = nwork_cm.__enter__()
                zwork_cm = tc.tile_pool(name="zwork", bufs=2)
                zwork = zwork_cm.__enter__()

                def open_scale(s):
                    Tf = L // s
                    nvt = Tf // 128
                    cm = tc.tile_pool(name=f"sc{s}", bufs=1)
                    scp = cm.__enter__()
                    K_sb = scp.tile([128, B, Tf], FP8, name=f"K{s}",
                                    tag=f"K{s}")
                    Q_sb = scp.tile([128, B, Tf], FP8, name=f"Q{s}",
                                    tag=f"Q{s}")
                    V_sb = scp.tile([128, B, nvt, HPC, HD + 2], FP8,
                                    name=f"V{s}", tag=f"V{s}")
                    _c2 = c8[:, 1:3]
                    nc.scalar.dma_start(
                        out=V_sb[:, :, :, :, HD:HD + 2],
                        in_=bass.AP(tensor=_c2.tensor, offset=_c2.offset,
                                    ap=[list(_c2.ap[0]),
                                        [0, B * nvt * HPC], [1, 2]]))
                    sendA = scp.tile([64, B, Tf], FP8,
                                     name=f"sA{s}", tag=f"sA{s}")
                    st[s] = dict(cm=cm, K=K_sb, Q=Q_sb, V=V_sb, sendA=sendA)

                def proj_units(s):
                    """Per-PSUM-tile projection emitters (closures)."""
                    si = SCALES.index(s)
                    Tf = L // s
                    xs = xs_sb[s]
                    wk_sb = qkv_w[s]["wk"]
                    wq_sb = qkv_w[s]["wq"]
                    wv_sb = qkv_w[s]["wv"]
                    K_sb, Q_sb, V_sb = st[s]["K"], st[s]["Q"], st[s]["V"]
                    units = []

                    def mk_kq(kind, bb, tt):
                        def emit():
                            ps = ppmm.tile([128, 512], F32, name="mm", tag="mm")
                            wsb = wk_sb if kind == "k" else wq_sb
                            for hp in range(4):
                                nc.tensor.matmul(
                                    ps[:, :],
                                    wsb[:, ds(hp * 2, 2), :],
                                    xs[:, ds(hp * 2, 2), bb, ds(tt * 512, 512)],
                                    start=(hp == 0), stop=(hp == 3),
                                    perf_mode=DR,
                                )
                            if kind == "k":
                                nc.vector.tensor_copy(
                                    out=K_sb[:, bb, ds(tt * 512, 512)],
                                    in_=ps[:, :])
                            else:
                                nc.vector.tensor_scalar_add(
                                    Q_sb[:, bb, ds(tt * 512, 512)], ps[:, :],
                                    bq_sb[:, ds(si, 1)])
                        return emit

                    def mk_v(bb, vt0, nv):
                        def emit():
                            ps = ppmm.tile([128, 512], F32, name="mm", tag="mm")
                            for j in range(nv):
                                for hc in range(8):
                                    nc.tensor.matmul(
                                        ps[:, ds(j * FPC, FPC)],
                                        xs[:, hc, bb, ds((vt0 + j) * 128, 128)],
                                        wv_sb[:, hc, :],
                                        start=(hc == 0), stop=(hc == 7),
                                    )
                            bvb = bvf_sb[:, si, :]
                            nc.vector.tensor_add(
                                V_sb[:, bb, ds(vt0, nv), :, 0:HD],
                                ps[:, 0:nv * FPC].rearrange(
                                    "p (v h d) -> p v h d", v=nv, d=HD),
                                bass.AP(tensor=bvb.tensor, offset=bvb.offset,
                                        ap=[list(bvb.ap[0]), [0, nv],
                                            [HD, HPC], [1, HD]]))
                        return emit

                    for bb in range(B):
                        for tt in range(Tf // 512):
                            units.append(mk_kq("k", bb, tt))
                        for tt in range(Tf // 512):
                            units.append(mk_kq("q", bb, tt))
                        for vt0 in range(0, Tf // 128, 4):
                            units.append(mk_v(bb, vt0, min(4, Tf // 128 - vt0)))
                    return units

                def ship(s, hl, bb):
                    To = TPC // s
                    sendA = st[s]["sendA"]
                    if s == 1:
                        dst = a2aB_in[ds(bb * GROUP, GROUP), hl, :, :]
                    else:
                        off = (0 if s == 2 else _SHA_S2) + hl * 64 * To
                        dst = a2aA_in[ds(bb * GROUP, GROUP),
                                      off:off + 64 * To].rearrange(
                                          "j (r t) -> j r t", r=64)
                    nc.gpsimd.dma_start(
                        out=dst.rearrange("j r t -> r j t"),
                        in_=sendA[:, bb, :].rearrange(
                            "r (j t) -> r j t", j=GROUP))

                def attn(s, hl, fill=None, fill_per_unit=0, fill_skip=0):
                    Tf = L // s
                    nkp = Tf // 256
                    K_sb, Q_sb, V_sb = st[s]["K"], st[s]["Q"], st[s]["V"]
                    sendA = st[s]["sendA"]
                    po = hl * 64
                    unit = 0
                    for bb in range(B):
                        for qt in range(Tf // 512):
                            ctx = ppctx.tile([65, 512], F32,
                                             name="ctx", tag="ctx")
                            pend = None
                            for kp in range(nkp):
                                sp = ppsp.tile([128, 1024], F32,
                                               name="sp", tag="sp")
                                for half in range(2):
                                    kt = kp * 2 + half
                                    nc.tensor.matmul(
                                        sp[:, ds(half * 512, 512)],
                                        _pair0(K_sb[ds(po, 64), bb,
                                                    ds(kt * 128, 128)]),
                                        _pair0(Q_sb[ds(po, 64), bb,
                                                    ds(qt * 512, 512)]),
                                        start=True, stop=True,
                                        perf_mode=DR,
                                    )
                                e = awork.tile([128, 2, 512], FP8,
                                               name="esb", tag="esb")
                                nc.scalar.activation(
                                    e.rearrange("p two q -> p (two q)"),
                                    sp[:, :], AF.Exp, scale=EXP_SCALE_ATT)
                                if pend is not None:
                                    _ctx_pair(nc, ctx, V_sb, pend[0],
                                              pend[1], bb, hl, nkp)
                                pend = (kp, e)
                            _ctx_pair(nc, ctx, V_sb, pend[0], pend[1],
                                      bb, hl, nkp)
                            zinv = nwork.tile([1, 512], F32R,
                                              name="zi", tag="zi")
                            with nc.allow_low_precision(reason="softmax norm"):
                                nc.vector.reciprocal(out=zinv,
                                                     in_=ctx[64:65, :])
                            zbp = ppmm.tile([64, 512], F32, name="mm", tag="mm")
                            nc.tensor.matmul(zbp[:, :], onesr_sb[:, 0:64],
                                             zinv[:, :], start=True, stop=True)
                            zb = zwork.tile([64, 512], BF16, name="zb", tag="zb")
                            nc.vector.tensor_copy(out=zb, in_=zbp[:, :])
                            nc.vector.tensor_mul(
                                sendA[:, bb, ds(qt * 512, 512)],
                                ctx[0:64, :], zb)
                            # drain interleaved projection work into the
                            # exp-bound stretch (PE queue is in-order)
                            if fill is not None and unit >= fill_skip:
                                for _ in range(fill_per_unit):
                                    if fill:
                                        fill.pop(0)()
                            unit += 1
                        ship(s, hl, bb)

                # pools close LIFO: sc1 opened first survives longest
                open_scale(1)
                open_scale(4)
                open_scale(2)
                for u in proj_units(2):
                    u()
                fill = proj_units(4) + proj_units(1)
                attn(2, 0, fill=fill, fill_per_unit=4, fill_skip=3)
                attn(2, 1, fill=fill, fill_per_unit=6, fill_skip=0)
                for u in fill:   # leftovers, if any
                    u()
                attn(4, 0)
                attn(4, 1)
                nc.gpsimd.collective_compute(
                    "AllToAll", mybir.AluOpType.bypass,
                    replica_groups=[list(range(NCORES))],
                    ins=[a2aA_in.opt()], outs=[a2aA_out.opt()])
                attn(1, 0)
                # receive A (scales 2+4) while scale-1 attention continues
                with tc.tile_pool(name="rcv", bufs=1) as rcv:
                    for s in (2, 4):
                        To = TPC // s
                        off0 = 0 if s == 2 else _SHA_S2
                        ctxR = rcv.tile([128, 8, To], FP8,
                                        name=f"ctxR{s}", tag=f"ctxR{s}")
                        for hl in range(HPC):
                            off = off0 + hl * 64 * To
                            src = a2aA_out[:, off:off + 64 * To].rearrange(
                                "k (r t) -> k r t", r=64)
                            nc.gpsimd.dma_start(
                                out=ctxR[ds(hl * 64, 64), :, :],
                                in_=src.rearrange("k r t -> r k t"))
                        eng = nc.vector if s == 2 else nc.gpsimd
                        eng.tensor_copy(
                            out=ctxU[s].rearrange(
                                "p tt k (u j) -> p tt k u j", j=s),
                            in_=ctxR.rearrange(
                                "p k (tt u) -> p tt k u", tt=4).unsqueeze(
                                    -1).broadcast_to([128, 4, 8, 128 // s, s]))
                    attn(1, 1)
                nc.gpsimd.collective_compute(
                    "AllToAll", mybir.AluOpType.bypass,
                    replica_groups=[list(range(NCORES))],
                    ins=[a2aB_in.opt()], outs=[a2aB_out.opt()])
                st[2]["cm"].__exit__(None, None, None)
                st[4]["cm"].__exit__(None, None, None)
                st[1]["cm"].__exit__(None, None, None)
                zwork_cm.__exit__(None, None, None)
                nwork_cm.__exit__(None, None, None)
                awork_cm.__exit__(None, None, None)

            # ---------- late phases: P3 memory, P1 gate, W1+LN ----------
            with tc.tile_pool(name="lateB", bufs=1) as lateB:
                xin_full = lateB.tile([128, 4, H], F32, name="xinf", tag="xinf")
                nc.sync.dma_start(
                    out=xin_full, in_=xt.rearrange("(tt p) h -> p tt h", p=128))
                reps = lateB.tile([128, 3, H], BF16, name="reps", tag="reps")
                _vb = vecsb[:, :]
                nc.sync.dma_start(
                    out=reps,
                    in_=bass.AP(tensor=_vb.tensor, offset=_vb.offset,
                                ap=[[0, 128]] + [list(p) for p in _vb.ap]))
                b2rep = lateB.tile([128, H], F32, name="b2rep", tag="b2rep")
                nc.sync.dma_start(out=b2rep, in_=rep_vec(1))
                scg = lateB.tile([128, 4], F32, name="scg", tag="scg")

                # P3: memory softmax (fp8 DR)
                with tc.tile_pool(name="memp", bufs=1) as memp, \
                     tc.tile_pool(name="mwork", bufs=1) as mwork:
                    Em_sb = memp.tile([128, 8, TPC], FP8)
                    for mt in range(8):
                        ps = ppmm.tile([128, TPC], F32, name="mm", tag="mm")
                        for hp in range(4):
                            nc.tensor.matmul(
                                ps[:, :],
                                emT_sb[:, mt, ds(hp * 2, 2), :],
                                xTo8_sb[:, ds(hp * 2, 2), :],
                                start=(hp == 0), stop=(hp == 3),
                                perf_mode=DR,
                            )
                        nc.scalar.activation(Em_sb[:, mt, :], ps[:, :],
                                             AF.Exp, scale=EXP_SCALE_MEM)
                    zps = ppmm.tile([32, TPC], F32, name="mm", tag="mm")
                    for mp in range(4):
                        nc.tensor.matmul(zps[:, :], zsel_sb[:, :, :],
                                         Em_sb[:, ds(mp * 2, 2), :],
                                         start=(mp == 0), stop=(mp == 3),
                                         perf_mode=DR)
                    zinv = mwork.tile([1, TPC], F32R, name="zim", tag="zim")
                    with nc.allow_low_precision(reason="softmax norm bcast"):
                        nc.vector.reciprocal(out=zinv, in_=zps[0:1, :])
                    zbp = ppmm.tile([128, TPC], F32, name="mm", tag="mm")
                    nc.tensor.matmul(zbp[:, :], onesr_sb[:, :], zinv[:, :],
                                     start=True, stop=True)
                    zb = mwork.tile([128, TPC], F32, name="zbm", tag="zbm")
                    nc.vector.tensor_copy(out=zb, in_=zbp[:, :])
                    zb2 = zb.rearrange("p (tt t) -> p tt t", t=128)
                    for ht in range(8):
                        ps = ppmm.tile([128, TPC], F32, name="mm", tag="mm")
                        for mp in range(4):
                            nc.tensor.matmul(
                                ps[:, :],
                                em_sb[:, ht, ds(mp * 2, 2), :],
                                Em_sb[:, ds(mp * 2, 2), :],
                                start=(mp == 0), stop=(mp == 3),
                                perf_mode=DR,
                            )
                        nc.vector.tensor_mul(
                            mcT8[:, :, ht, :],
                            ps.rearrange("p (tt t) -> p tt t", t=128), zb2)

                # P1: error gate (bf16)
                with tc.tile_pool(name="gwork", bufs=2) as gwork:
                    for tt in range(4):
                        gf = gwork.tile([128, H], F32, name="gatef", tag="gatef")
                        for jh in range(2):
                            ps = ppmm.tile([128, 512], F32, name="mm", tag="mm")
                            for hc in range(8):
                                nc.tensor.matmul(
                                    ps[:, :],
                                    xTo_sb[:, hc, ds(tt * 128, 128)],
                                    w2_sb[:, hc, ds(jh * 512, 512)],
                                    start=(hc == 0), stop=(hc == 7),
                                )
                            lg = gwork.tile([128, 512], F32, name="lg", tag="lg")
                            nc.vector.tensor_add(
                                lg, ps[:, :], b2rep[:, ds(jh * 512, 512)])
                            nc.scalar.activation(lg, lg, AF.Tanh, scale=0.5)
                            nc.vector.tensor_scalar(
                                out=gf[:, ds(jh * 512, 512)], in0=lg,
                                scalar1=0.5, scalar2=0.5, op0=ALU.mult,
                                op1=ALU.add,
                            )
                        nc.vector.tensor_copy(out=gate_bf[:, tt, :], in_=gf)
                        nc.vector.reduce_sum(out=sumg[:, ds(tt, 1)], in_=gf,
                                             axis=mybir.AxisListType.X)
                        nc.sync.dma_start(out=out_g[ds(tt * 128, 128), :],
                                          in_=gf)
                # hoisted gate scale: sc = 1/sqrt(sumg + eps)
                nc.scalar.activation(scg, sumg, AF.Sqrt, bias=epst[:, :])
                nc.vector.reciprocal(out=scg, in_=scg)

                # W1 + LN + final combine.  Units (tt, jh); the first 6 units
                # pre-accumulate their s2/s4/mem chunks into dedicated PSUM
                # banks during the AllToAll-B flight; the s1 chunks land after
                # the receive.
                with tc.tile_pool(name="w1acc", bufs=1, space="PSUM") as w1acc, \
                     tc.tile_pool(name="fwork", bufs=4) as fwork:
                    units = [(tt, jh) for tt in range(4) for jh in range(2)]
                    early = units[:6]
                    psu = {}

                    def w1_mm(ps, tt, jh, which, nmm, total):
                        for si, s in enumerate(SCALES):
                            if (s == 1) != (which == "s1"):
                                continue
                            for hp in range(4):
                                nc.tensor.matmul(
                                    ps[:, :],
                                    ctxU[s][:, tt, ds(hp * 2, 2), :],
                                    wf_sb[si][:, ds(hp * 2, 2),
                                              ds(jh * 512, 512)],
                                    start=(nmm == 0), stop=(nmm == total - 1),
                                    perf_mode=DR,
                                )
                                nmm += 1
                        if which != "s1":
                            for hp in range(4):
                                nc.tensor.matmul(
                                    ps[:, :],
                                    mcT8[:, tt, ds(hp * 2, 2), :],
                                    w1m_sb[:, ds(hp * 2, 2), ds(jh * 512, 512)],
                                    start=(nmm == 0), stop=(nmm == total - 1),
                                    perf_mode=DR,
                                )
                                nmm += 1
                        return nmm

                    for ui, (tt, jh) in enumerate(early):
                        ps = w1acc.tile([128, 512], F32, name=f"w1p{ui % 6}",
                                        tag=f"w1p{ui % 6}")
                        psu[(tt, jh)] = ps
                        w1_mm(ps, tt, jh, "early", 0, 16)

                    # receive B (scale 1), split so early tt finals start
                    # as soon as their token range lands
                    for tt in range(4):
                        for hl in range(HPC):
                            nc.gpsimd.dma_start(
                                out=ctxU[1][ds(hl * 64, 64), tt, :, :],
                                in_=a2aB_out[:, hl, :,
                                             ds(tt * 128, 128)].rearrange(
                                                 "k r t -> r k t"))

                    for tt in range(4):
                        cfr = fwork.tile([128, H], BF16, name="cfr", tag="cfr")
                        stats = fwork.tile([128, 2, 6], F32,
                                           name="stats", tag="stats")
                        for jh in range(2):
                            if (tt, jh) in psu:
                                ps = psu[(tt, jh)]
                                w1_mm(ps, tt, jh, "s1", 12, 16)
                            else:
                                ps = w1acc.tile([128, 512], F32,
                                                name=f"w1p{(tt * 2 + jh) % 6}",
                                                tag=f"w1p{(tt * 2 + jh) % 6}")
                                n = w1_mm(ps, tt, jh, "early", 0, 16)
                                w1_mm(ps, tt, jh, "s1", n, 16)
                            nc.vector.scalar_tensor_tensor(
                                out=cfr[:, ds(jh * 512, 512)], in0=ps[:, :],
                                scalar=PS_DESCALE,
                                in1=reps[:, 0, ds(jh * 512, 512)],
                                op0=ALU.mult, op1=ALU.add,
                            )
                            nc.vector.bn_stats(
                                out=stats[:, jh, :],
                                in_=cfr[:, ds(jh * 512, 512)])
                        mv = fwork.tile([128, 2], F32, name="mv", tag="mv")
                        nc.vector.bn_aggr(out=mv, in_=stats)
                        rstd = fwork.tile([128, 1], F32, name="rstd", tag="rstd")
                        nc.scalar.activation(rstd, mv[:, 1:2], AF.Sqrt,
                                             bias=epst[:, :])
                        nc.vector.reciprocal(out=rstd, in_=rstd)
                        nb = fwork.tile([128, 1], F32, name="nb", tag="nb")
                        nc.vector.scalar_tensor_tensor(
                            out=nb, in0=mv[:, 0:1], scalar=-1.0, in1=rstd,
                            op0=ALU.mult, op1=ALU.mult,
                        )
                        cf = fwork.tile([128, H], BF16, name="cf", tag="cf")
                        nc.vector.tensor_scalar(
                            out=cf, in0=cfr, scalar1=rstd[:, :],
                            scalar2=nb[:, :],
                            op0=ALU.mult, op1=ALU.add,
                        )
                        nc.gpsimd.tensor_mul(cf, cf, reps[:, 1, :])
                        nc.gpsimd.tensor_add(cf, cf, reps[:, 2, :])
                        nc.gpsimd.tensor_scalar_max(cf, cf, 0.0)
                        nc.vector.tensor_mul(cf, gate_bf[:, tt, :], cf)
                        yout = fwork.tile([128, H], BF16, name="yout", tag="yout")
                        nc.vector.scalar_tensor_tensor(
                            out=yout, in0=cf, scalar=scg[:, ds(tt, 1)],
                            in1=xin_full[:, tt, :],
                            op0=ALU.mult, op1=ALU.add,
                        )
                        nc.sync.dma_start(out=out_y[ds(tt * 128, 128), :],
                                          in_=yout)
            late_cm.__exit__(None, None, None)

    return _split_waits(nc)


def _ctx_pair(nc, ctx, V_sb, kp, e, bb, hl, nkp):
    for half in range(2):
        kt = kp * 2 + half
        nc.tensor.matmul(
            ctx[:, :],
            V_sb[:, bb, kt, hl, 0:HD + 1],
            e[:, half, :],
            start=(kp == 0 and half == 0),
            stop=(kp == nkp - 1 and half == 1),
        )


_CACHE = {}


def _get_program():
    if "nc" not in _CACHE:
        _CACHE["nc"] = _build_program()
    return _CACHE["nc"]


def _chunk8(a):
    """[1024, N] -> [128, 8, N] pre-chunked (partition-major) contiguous."""
    k, n = a.shape
    assert k == 1024
    return np.ascontiguousarray(a.reshape(8, 128, n).transpose(1, 0, 2))


def _prep_host(inputs):
    x = np.asarray(inputs["x"], np.float32)
    emx = np.asarray(inputs["error_memory"], np.float32)
    Wq = np.asarray(inputs["Wq"], np.float32)
    Wk = np.asarray(inputs["Wk"], np.float32)
    Wv = np.asarray(inputs["Wv"], np.float32)
    Wo = np.asarray(inputs["Wo"], np.float32)
    W1 = np.asarray(inputs["W1"], np.float32)
    W2 = np.asarray(inputs["W2"], np.float32)
    bq = np.asarray(inputs["bq"], np.float32)
    bv = np.asarray(inputs["bv"], np.float32)
    bo = np.asarray(inputs["bo"], np.float32)
    b1 = np.asarray(inputs["b1"], np.float32)
    b2 = np.asarray(inputs["b2"], np.float32)
    lns = np.asarray(inputs["ln_scale"], np.float32)
    lnb = np.asarray(inputs["ln_bias"], np.float32)

    # host-side multi-scale mean pooling, feature-major, fp8, pre-chunked
    x2 = x.reshape(B, L // 2, 2, H).mean(axis=2)
    x4 = x.reshape(B, L // 4, 4, H).mean(axis=2)

    def chunkx(a):  # [B, T, H] -> [128, 8, B, T] fp8
        t = a.shape[1]
        aT = a.transpose(2, 0, 1).reshape(8, 128, B, t)
        return np.ascontiguousarray(aT.transpose(1, 0, 2, 3)).astype(E4)

    x1T_h = chunkx(x)
    x2T_h = chunkx(x2)
    x4T_h = chunkx(x4)

    wf_h = np.stack([_chunk8(64.0 * (Wo[i] @ W1[i * H:(i + 1) * H]))
                     for i in range(3)]).astype(E4)
    w1m_h = _chunk8(SW * W1[3 * H:4 * H]).astype(E4)
    w2_h = _chunk8(W2).astype(BF)
    def _chunk88(a):  # [1024, 1024] -> [128, 8(out-tile), 8(chunk), 128]
        c = _chunk8(a)          # [128, 8chunk, 1024]
        return np.ascontiguousarray(
            c.reshape(128, 8, 8, 128).transpose(0, 2, 1, 3))
    emT_h = _chunk88(SW * np.ascontiguousarray(emx.T)).astype(E4)
    em_h = _chunk88(SW * emx).astype(E4)
    zsel_h = np.zeros((128, 64), np.float32)
    zsel_h[:, 0] = 1.0
    zsel_h[:, 32] = 1.0
    zsel_h = zsel_h.astype(E4)
    b1e = b1 + sum(bo[i] @ W1[i * H:(i + 1) * H] for i in range(3))
    vecs_h = np.stack([b1e, b2, lns, lnb]).astype(np.float32)
    vecsb_h = np.stack([b1e, lns, lnb]).astype(BF)

    shared = dict(wf=wf_h, w1m=w1m_h, w2=w2_h, emT=emT_h, em=em_h, zsel=zsel_h,
                  vecs=vecs_h, vecsb=vecsb_h,
                  x1T=x1T_h, x2T=x2T_h, x4T=x4T_h,
                  onesr=np.ones((1, 128), np.float32),
                  c8=np.concatenate([np.ones((128, 1), np.float32),
                                     np.full((128, 1), 2.0, np.float32),
                                     np.zeros((128, 1), np.float32)],
                                    axis=1).astype(E4))

    in_maps = []
    for c in range(NCORES):
        b, g = divmod(c, GROUP)
        sl = x[b, g * TPC:(g + 1) * TPC]
        m = dict(shared)
        m["xt"] = np.ascontiguousarray(sl).astype(np.float32)
        slT = np.ascontiguousarray(sl.T)
        m["xToT"] = _chunk8(slT).astype(BF)
        m["xToT8"] = _chunk8(slT).astype(E4)
        fs = slice(c * FPC, (c + 1) * FPC)
        m["wq"] = np.stack([_chunk8(SW * Wq[i][:, fs]) for i in range(3)]
                           ).astype(E4)
        m["wk"] = np.stack([_chunk8(SW * Wk[i][:, fs]) for i in range(3)]
                           ).astype(E4)
        m["wv"] = np.stack([_chunk8(SW * Wv[i][:, fs]) for i in range(3)]
                           ).astype(E4)
        m["bqs"] = np.ascontiguousarray(SW * bq[:, fs].T)  # [128, 3]
        m["bvf"] = np.ascontiguousarray(SW * bv[:, fs])    # [3, FPC]
        in_maps.append(m)
    return in_maps


def _runner():
    """Build (once) a cached jitted 8-core executable for this program."""
    if "run" in _CACHE:
        return _CACHE["run"]
    import jax
    from jax.experimental.shard_map import shard_map
    from jax.sharding import Mesh, PartitionSpec
    from concourse import bass2jax

    nc = _get_program()
    bass2jax.install_neuronx_cc_hook()
    partition_name = (nc.partition_id_tensor.name
                      if nc.partition_id_tensor else None)
    in_names, out_names, out_avals = [], [], []
    for alloc in nc.m.functions[0].allocations:
        if not isinstance(alloc, mybir.MemoryLocationSet):
            continue
        name = alloc.memorylocations[0].name
        if alloc.kind == "ExternalInput":
            if name != partition_name:
                in_names.append(name)
        elif alloc.kind == "ExternalOutput":
            out_names.append(name)
            out_avals.append(jax.core.ShapedArray(
                tuple(alloc.tensor_shape), mybir.dt.np(alloc.dtype)))
    n_params = len(in_names)
    n_outs = len(out_avals)
    all_in = list(in_names) + list(out_names)
    if partition_name is not None:
        all_in.append(partition_name)
    donate = tuple(range(n_params, n_params + n_outs))

    def _body(*args):
        operands = list(args)
        if partition_name is not None:
            operands.append(bass2jax.partition_id_tensor())
        outs = bass2jax._bass_exec_p.bind(
            *operands,
            out_avals=tuple(out_avals),
            in_names=tuple(all_in),
            out_names=tuple(out_names),
            lowering_input_output_aliases=(),
            sim_require_finite=True,
            sim_require_nnan=True,
            nc=nc,
        )
        return tuple(outs)

    devices = jax.devices()[:NCORES]
    mesh = Mesh(np.asarray(devices), ("core",))
    in_specs = (PartitionSpec("core"),) * (n_params + n_outs)
    out_specs = (PartitionSpec("core"),) * n_outs
    fn = jax.jit(
        shard_map(_body, mesh=mesh, in_specs=in_specs,
                  out_specs=out_specs, check_rep=False),
        donate_argnums=donate, keep_unused=True)
    _CACHE["run"] = (fn, in_names, out_names, out_avals, mesh)
    return _CACHE["run"]


def _concat_inputs(in_maps, in_names):
    return [np.concatenate([np.asarray(in_maps[c][n]) for c in range(NCORES)],
                           axis=0) for n in in_names]


def kernel(**inputs):
    fn, in_names, out_names, out_avals, mesh = _runner()
    in_maps = _prep_host(inputs)
    concat_in = _concat_inputs(in_maps, in_names)
    zeros = [np.zeros((NCORES * a.shape[0], *a.shape[1:]), a.dtype)
             for a in out_avals]
    outs = fn(*concat_in, *zeros)
    res = {n: np.asarray(outs[i]) for i, n in enumerate(out_names)}
    y = res["out_y"].astype(np.float32).reshape(B, L, H)
    g = res["out_g"].reshape(B, L, H)
    return y, g
